# revision 21
# baseline (speedup 1.0000x reference)
"""Trainium2 Bass kernel for DirectMPIGO volumetric rendering (segment_reduce).

reference math (interval == 1):
    x               = density + shift
    one_minus_alpha = sigmoid(-x)             # = exp(-softplus(x))
    T_incl          = cumprod(one_minus_alpha, axis=1)
    weights         = T_excl - T_incl         # == T_excl * alpha
    alphainv_last   = T_incl[:, -1]
    rgb_marched     = einsum('ns,nsc->nc', weights, rgb) + alphainv_last[:,None]*bg

Sharding: rays (axis 0) split evenly across the 8 cores, no communication.

Per-core pipeline (4096 rays = 8 groups x 4 sub-tiles x 128 rays):
    SP   : input DMAs (density, rgb) + weights store DMAs (HWDGE)
    Pool : x = density+shift; w = T_excl - T_incl (+ boundary cols, T_last)
    ACT  : u = sigmoid(-x) in place
    DVE  : per-ray cumprod scans; rgb einsum via scalar_tensor_tensor accum
Small outputs are accumulated in SBUF [128, TILES] channel-planar, block
transposed at the end, and stored with >=512B-contiguous DMA runs (the
host re-interleaves rgb_marched from [3, N] to [N, 3]).
"""

from contextlib import ExitStack

import numpy as np

N_RAYS = 32768
S = 256
N_CORES = 8
N_PER_CORE = N_RAYS // N_CORES  # 4096
P = 128
TILES = N_PER_CORE // P  # 32 ray-tiles per core
G = 4  # ray-tiles per group
GROUPS = TILES // G  # 8
IN_BUFS = 6
W_BUFS = 4

LAST_EXEC_TIME_NS = None
LAST_RESULT = None

_program_cache = {}


def _build_program(interval: float):
    import concourse.bass as bass
    from concourse import mybir

    f32 = mybir.dt.float32
    Alu = mybir.AluOpType
    Act = mybir.ActivationFunctionType

    nc = bass.Bass()
    density_h = nc.declare_dram_parameter("density", [N_PER_CORE, S], f32, isOutput=False)
    shift_h = nc.declare_dram_parameter("shift", [S], f32, isOutput=False)
    rgb_h = nc.declare_dram_parameter("rgb", [N_PER_CORE, S * 3], f32, isOutput=False)
    bg_h = nc.declare_dram_parameter("bg", [3], f32, isOutput=False)
    rgbm_h = nc.declare_dram_parameter("rgb_marched_t", [3, N_PER_CORE], f32, isOutput=True)
    weights_h = nc.declare_dram_parameter("weights", [N_PER_CORE, S], f32, isOutput=True)
    alphainv_h = nc.declare_dram_parameter("alphainv_last", [N_PER_CORE], f32, isOutput=True)

    GS = G * S  # 1024 free elems per group (density/weights)
    B = 32  # DVE stream-transpose block size

    def bcast_ap(h):
        ap = h[:]
        return bass.AP(
            tensor=ap.tensor,
            offset=ap.offset,
            ap=[[0, P]] + [list(p) for p in ap.ap],
        )

    def d_dram(g):
        return (
            density_h[g * G * P : (g + 1) * G * P, :]
            .rearrange("(j p) s -> j p s", p=P)
            .transpose([1, 0, 2])
        )

    def r_dram(g):
        return (
            rgb_h[g * G * P : (g + 1) * G * P, :]
            .rearrange("(j p) s -> j p s", p=P)
            .transpose([1, 0, 2])
        )

    def w_dram(g):
        return (
            weights_h[g * G * P : (g + 1) * G * P, :]
            .rearrange("(j p) s -> j p s", p=P)
            .transpose([1, 0, 2])
        )

    with ExitStack() as ctx:
        d_bufs = [
            ctx.enter_context(nc.sbuf_tensor(f"d_buf{i}", [P, GS], f32))
            for i in range(IN_BUFS)
        ]
        r_bufs = [
            ctx.enter_context(nc.sbuf_tensor(f"r_buf{i}", [P, 3 * GS], f32))
            for i in range(IN_BUFS)
        ]
        w_bufs = [
            ctx.enter_context(nc.sbuf_tensor(f"w_buf{i}", [P, GS], f32))
            for i in range(W_BUFS)
        ]
        # t layout per sub-tile: [1.0, T[0..255]] -- the leading ones column
        # (memset once, never rewritten) lets one 3D subtract produce every
        # weight column, boundary included.
        t_bufs = [
            ctx.enter_context(nc.sbuf_tensor(f"t_buf{i}", [P, G * (S + 1)], f32))
            for i in range(2)
        ]
        scratch = ctx.enter_context(nc.psum_tensor([P, S], f32))
        shiftb = ctx.enter_context(nc.sbuf_tensor([P, S], f32))
        bgb = ctx.enter_context(nc.sbuf_tensor([P, 3], f32))
        abuf = ctx.enter_context(nc.sbuf_tensor([P, TILES], f32))
        rm_all = ctx.enter_context(nc.sbuf_tensor([P, 3 * TILES], f32))
        at_t = ctx.enter_context(nc.sbuf_tensor([TILES, P], f32))
        rm_t = ctx.enter_context(nc.sbuf_tensor([TILES, 3 * P], f32))

        s_ind = ctx.enter_context(nc.semaphore("s_ind"))    # density loads done
        s_inr = ctx.enter_context(nc.semaphore("s_inr"))    # rgb loads done
        s_add = ctx.enter_context(nc.semaphore("s_add"))    # Pool add done
        s_u = ctx.enter_context(nc.semaphore("s_u"))        # ACT sigmoid done
        s_t = ctx.enter_context(nc.semaphore("s_t"))        # DVE scans done
        s_w = ctx.enter_context(nc.semaphore("s_w"))        # Pool w-production done
        s_tc = ctx.enter_context(nc.semaphore("s_tc"))      # Pool done reading t_buf
        s_wst = ctx.enter_context(nc.semaphore("s_wst"))    # weight store DMAs
        s_dve = ctx.enter_context(nc.semaphore("s_dve"))    # DVE einsum done
        s_bc = ctx.enter_context(nc.semaphore("s_bc"))      # shift broadcast DMA
        s_bg = ctx.enter_context(nc.semaphore("s_bg"))      # bg broadcast DMA
        s_fin = ctx.enter_context(nc.semaphore("s_fin"))    # DVE final section done
        s_fout = ctx.enter_context(nc.semaphore("s_fout"))  # final output DMAs

        block = ctx.enter_context(nc.Block())

        @block.sync
        def _(sync):
            def load_d(g):
                if g >= IN_BUFS:
                    sync.wait_ge(s_t, g - IN_BUFS + 1)     # d_buf free (scans done)
                sync.dma_start(out=d_bufs[g % IN_BUFS][:], in_=d_dram(g)).then_inc(
                    s_ind, 16
                )

            def load_r(g):
                if g >= IN_BUFS:
                    sync.wait_ge(s_dve, g - IN_BUFS + 1)   # r_buf free (einsum done)
                sync.dma_start(out=r_bufs[g % IN_BUFS][:], in_=r_dram(g)).then_inc(
                    s_inr, 16
                )

            load_d(0)
            load_d(1)
            for g in range(GROUPS - 2):
                load_r(g)
                load_d(g + 2)
            load_r(GROUPS - 2)
            load_r(GROUPS - 1)


        @block.scalar
        def _(scalar):
            for g in range(GROUPS):
                scalar.wait_ge(s_add, g + 1)
                d_g = d_bufs[g % IN_BUFS]
                if interval == 1.0:
                    scalar.activation(d_g[:], d_g[:], Act.Sigmoid, scale=-1.0)
                else:
                    scalar.activation(d_g[:], d_g[:], Act.Sigmoid, scale=-1.0)
                    scalar.activation(d_g[:], d_g[:], Act.Ln)
                    scalar.activation(d_g[:], d_g[:], Act.Exp, scale=float(interval))
                scalar.drain().then_inc(s_u, 1)
                if g >= 1:
                    scalar.wait_ge(s_w, g)
                    scalar.dma_start(
                        out=w_dram(g - 1), in_=w_bufs[(g - 1) % W_BUFS][:]
                    ).then_inc(s_wst, 16)
            scalar.wait_ge(s_w, GROUPS)
            scalar.dma_start(
                out=w_dram(GROUPS - 1), in_=w_bufs[(GROUPS - 1) % W_BUFS][:]
            ).then_inc(s_wst, 16)

        @block.gpsimd
        def _(gpsimd):
            gpsimd.dma_start(out=shiftb[:], in_=bcast_ap(shift_h)).then_inc(s_bc, 16)
            gpsimd.dma_start(out=bgb[:], in_=bcast_ap(bg_h)).then_inc(s_bg, 16)
            gpsimd.wait_ge(s_wst, 16 * GROUPS)
            gpsimd.wait_ge(s_fin, 1)
            gpsimd.dma_start(
                out=alphainv_h[:].rearrange("(t p) -> t p", p=P), in_=at_t[:]
            ).then_inc(s_fout, 16)
            gpsimd.dma_start(
                out=rgbm_h[:].rearrange("c (t p) -> t c p", p=P),
                in_=rm_t[:].rearrange("t (c p) -> t c p", p=P),
            ).then_inc(s_fout, 16)
            gpsimd.wait_ge(s_fout, 32)

        @block.vector
        def _(vector):
            def add_group(g):
                d_g = d_bufs[g % IN_BUFS]
                xv = d_g[:].rearrange("p (j s) -> p j s", j=G)
                vector.tensor_tensor(
                    out=xv,
                    in0=xv,
                    in1=shiftb[:, None, :].broadcast_to([P, G, S]),
                    op=Alu.add,
                ).then_inc(s_add, 1)

            def scans(g):
                d_g = d_bufs[g % IN_BUFS]
                t_g = t_bufs[g % 2]
                for j in range(G):
                    inst = vector.tensor_tensor_scan(
                        out=t_g[:, j * (S + 1) + 1 : (j + 1) * (S + 1)],
                        data0=d_g[:, j * S : (j + 1) * S],
                        data1=d_g[:, j * S : (j + 1) * S],
                        initial=1.0,
                        op0=Alu.mult,
                        op1=Alu.bypass,
                    )
                    if j == G - 1:
                        inst.then_inc(s_t, 1)

            vector.memset(t_bufs[0][:], 1.0)
            vector.memset(t_bufs[1][:], 1.0)

            vector.wait_ge(s_bc, 16)
            vector.wait_ge(s_ind, 16)
            add_group(0)
            vector.wait_ge(s_u, 1)
            scans(0)

            for g in range(GROUPS):
                if g + 1 < GROUPS:
                    vector.wait_ge(s_ind, 16 * (g + 2))
                    add_group(g + 1)

                # weights = T_excl - T_incl from this group's cumprod
                t_g = t_bufs[g % 2]
                w_g = w_bufs[g % W_BUFS]
                tv = t_g[:].rearrange("p (j s1) -> p j s1", j=G)
                wv = w_g[:].rearrange("p (j s) -> p j s", j=G)
                if g >= W_BUFS:
                    vector.wait_ge(s_wst, 16 * (g - W_BUFS + 1))
                vector.tensor_tensor(
                    out=wv,
                    in0=tv[:, :, 0:S],
                    in1=tv[:, :, 1 : S + 1],
                    op=Alu.subtract,
                )
                vector.tensor_copy(
                    out=abuf[:, g * G : (g + 1) * G], in_=tv[:, :, S]
                )

                if g + 1 < GROUPS:
                    vector.wait_ge(s_u, g + 2)
                    scans(g + 1)

                vector.wait_ge(s_inr, 16 * (g + 1))
                r_g = r_bufs[g % IN_BUFS]
                wv = w_g[:].rearrange("p (j s) -> p j s", j=G)
                rv = r_g[:].rearrange("p (j s c) -> p j s c", j=G, c=3)
                for j in range(G):
                    for c in range(3):
                        t_glob = g * G + j
                        inst = nc.vector.scalar_tensor_tensor(
                            out=scratch[:],
                            in0=wv[:, j, :],
                            scalar=0.0,
                            in1=rv[:, j, :, c],
                            op0=Alu.bypass,
                            op1=Alu.mult,
                            accum_out=rm_all[
                                :, c * TILES + t_glob : c * TILES + t_glob + 1
                            ],
                        )
                        if j == 0 and c == 0:
                            inst.then_inc(s_w, 1)
                        if j == G - 1 and c == 2:
                            inst.then_inc(s_dve, 1)

            vector.wait_ge(s_bg, 16)
            # rgb_marched += alphainv_last * bg  (channel-planar, contiguous)
            for c in range(3):
                nc.vector.scalar_tensor_tensor(
                    out=rm_all[:, c * TILES : (c + 1) * TILES],
                    in0=abuf[:],
                    scalar=bgb[:, c : c + 1],
                    in1=rm_all[:, c * TILES : (c + 1) * TILES],
                    op0=Alu.mult,
                    op1=Alu.add,
                )

            # transpose abuf [P, TILES] -> at_t [TILES, P] in 32x32 blocks
            for i in range(P // B):
                vector.transpose(
                    out=at_t[0:B, i * B : (i + 1) * B],
                    in_=abuf[i * B : (i + 1) * B, :],
                )
            # transpose each channel plane [P, TILES] -> [TILES, P] blockwise
            for c in range(3):
                for i in range(P // B):
                    vector.transpose(
                        out=rm_t[0:B, c * P + i * B : c * P + (i + 1) * B],
                        in_=rm_all[i * B : (i + 1) * B, c * TILES : (c + 1) * TILES],
                    )
            vector.drain().then_inc(s_fin, 1)

    return nc


def kernel(density, shift, rgb, bg, interval):
    global LAST_EXEC_TIME_NS, LAST_RESULT
    from concourse.bass_utils import run_bass_kernel_spmd

    density = np.ascontiguousarray(density, dtype=np.float32)
    shift = np.ascontiguousarray(shift, dtype=np.float32)
    rgb = np.ascontiguousarray(rgb, dtype=np.float32)
    bg = np.ascontiguousarray(bg, dtype=np.float32)
    interval_f = float(np.asarray(interval))

    key = interval_f
    if key not in _program_cache:
        _program_cache[key] = _build_program(interval_f)
    nc = _program_cache[key]

    rgb2 = rgb.reshape(N_RAYS, S * 3)
    in_maps = []
    for i in range(N_CORES):
        rows = slice(i * N_PER_CORE, (i + 1) * N_PER_CORE)
        in_maps.append(
            {
                "density": density[rows],
                "shift": shift,
                "rgb": rgb2[rows],
                "bg": bg,
            }
        )

    res = run_bass_kernel_spmd(nc, in_maps, list(range(N_CORES)))
    LAST_EXEC_TIME_NS = res.exec_time_ns
    LAST_RESULT = res

    rgb_marched = np.concatenate(
        [np.ascontiguousarray(r["rgb_marched_t"].T) for r in res.results], axis=0
    )
    weights = np.concatenate([r["weights"] for r in res.results], axis=0)
    alphainv_last = np.concatenate([r["alphainv_last"] for r in res.results], axis=0)
    return rgb_marched, weights, alphainv_last


# revision 22
# speedup vs baseline: 1.0209x; 1.0209x over previous
"""Trainium2 Bass kernel for DirectMPIGO volumetric rendering (segment_reduce).

reference math (interval == 1):
    x               = density + shift
    one_minus_alpha = sigmoid(-x)             # = exp(-softplus(x))
    T_incl          = cumprod(one_minus_alpha, axis=1)
    weights         = T_excl - T_incl         # == T_excl * alpha
    alphainv_last   = T_incl[:, -1]
    rgb_marched     = einsum('ns,nsc->nc', weights, rgb) + alphainv_last[:,None]*bg

Sharding: rays (axis 0) split evenly across the 8 cores, no communication.

Per-core pipeline (4096 rays = 8 groups x 4 sub-tiles x 128 rays):
    SP   : input DMAs (density, rgb) + weights store DMAs (HWDGE)
    Pool : x = density+shift; w = T_excl - T_incl (+ boundary cols, T_last)
    ACT  : u = sigmoid(-x) in place
    DVE  : per-ray cumprod scans; rgb einsum via scalar_tensor_tensor accum
Small outputs are accumulated in SBUF [128, TILES] channel-planar, block
transposed at the end, and stored with >=512B-contiguous DMA runs (the
host re-interleaves rgb_marched from [3, N] to [N, 3]).
"""

from contextlib import ExitStack

import numpy as np

N_RAYS = 32768
S = 256
N_CORES = 8
N_PER_CORE = N_RAYS // N_CORES  # 4096
P = 128
TILES = N_PER_CORE // P  # 32 ray-tiles per core
G = 4  # ray-tiles per group
GROUPS = TILES // G  # 8
IN_BUFS = 6
W_BUFS = 4

LAST_EXEC_TIME_NS = None
LAST_RESULT = None

_program_cache = {}


def _build_program(interval: float):
    import concourse.bass as bass
    from concourse import mybir

    f32 = mybir.dt.float32
    Alu = mybir.AluOpType
    Act = mybir.ActivationFunctionType

    nc = bass.Bass()
    density_h = nc.declare_dram_parameter("density", [N_PER_CORE, S], f32, isOutput=False)
    shift_h = nc.declare_dram_parameter("shift", [S], f32, isOutput=False)
    rgb_h = nc.declare_dram_parameter("rgb", [N_PER_CORE, S * 3], f32, isOutput=False)
    bg_h = nc.declare_dram_parameter("bg", [3], f32, isOutput=False)
    rgbm_h = nc.declare_dram_parameter("rgb_marched_t", [3, N_PER_CORE], f32, isOutput=True)
    weights_h = nc.declare_dram_parameter("weights", [N_PER_CORE, S], f32, isOutput=True)
    alphainv_h = nc.declare_dram_parameter("alphainv_last", [N_PER_CORE], f32, isOutput=True)

    GS = G * S  # 1024 free elems per group (density/weights)
    B = 32  # DVE stream-transpose block size

    def bcast_ap(h):
        ap = h[:]
        return bass.AP(
            tensor=ap.tensor,
            offset=ap.offset,
            ap=[[0, P]] + [list(p) for p in ap.ap],
        )

    def d_dram(g):
        return (
            density_h[g * G * P : (g + 1) * G * P, :]
            .rearrange("(j p) s -> j p s", p=P)
            .transpose([1, 0, 2])
        )

    def r_dram(g):
        return (
            rgb_h[g * G * P : (g + 1) * G * P, :]
            .rearrange("(j p) s -> j p s", p=P)
            .transpose([1, 0, 2])
        )

    def w_dram(g):
        return (
            weights_h[g * G * P : (g + 1) * G * P, :]
            .rearrange("(j p) s -> j p s", p=P)
            .transpose([1, 0, 2])
        )

    with ExitStack() as ctx:
        # density/u live in the same 257-wide segmented layout as t:
        # col j*257 is a pad column (0 in u, segment-restart marker), cols
        # j*257+1..j*257+256 hold the data for sub-tile j.
        d_bufs = [
            ctx.enter_context(nc.sbuf_tensor(f"d_buf{i}", [P, G * (S + 1)], f32))
            for i in range(IN_BUFS)
        ]
        r_bufs = [
            ctx.enter_context(nc.sbuf_tensor(f"r_buf{i}", [P, 3 * GS], f32))
            for i in range(IN_BUFS)
        ]
        w_bufs = [
            ctx.enter_context(nc.sbuf_tensor(f"w_buf{i}", [P, GS], f32))
            for i in range(W_BUFS)
        ]
        # t layout per sub-tile: [1.0, T[0..255]] -- the leading ones column
        # (memset once, never rewritten) lets one 3D subtract produce every
        # weight column, boundary included.
        t_bufs = [
            ctx.enter_context(nc.sbuf_tensor(f"t_buf{i}", [P, G * (S + 1)], f32))
            for i in range(2)
        ]
        scratch = ctx.enter_context(nc.psum_tensor([P, S], f32))
        z1 = ctx.enter_context(nc.sbuf_tensor([P, G * (S + 1)], f32))
        shiftb = ctx.enter_context(nc.sbuf_tensor([P, S], f32))
        bgb = ctx.enter_context(nc.sbuf_tensor([P, 3], f32))
        abuf = ctx.enter_context(nc.sbuf_tensor([P, TILES], f32))
        rm_all = ctx.enter_context(nc.sbuf_tensor([P, 3 * TILES], f32))
        at_t = ctx.enter_context(nc.sbuf_tensor([TILES, P], f32))
        rm_t = ctx.enter_context(nc.sbuf_tensor([TILES, 3 * P], f32))

        s_ind = ctx.enter_context(nc.semaphore("s_ind"))    # density loads done
        s_inr = ctx.enter_context(nc.semaphore("s_inr"))    # rgb loads done
        s_add = ctx.enter_context(nc.semaphore("s_add"))    # Pool add done
        s_u = ctx.enter_context(nc.semaphore("s_u"))        # ACT sigmoid done
        s_t = ctx.enter_context(nc.semaphore("s_t"))        # DVE scans done
        s_w = ctx.enter_context(nc.semaphore("s_w"))        # Pool w-production done
        s_tc = ctx.enter_context(nc.semaphore("s_tc"))      # Pool done reading t_buf
        s_wst = ctx.enter_context(nc.semaphore("s_wst"))    # weight store DMAs
        s_dve = ctx.enter_context(nc.semaphore("s_dve"))    # DVE einsum done
        s_bc = ctx.enter_context(nc.semaphore("s_bc"))      # shift broadcast DMA
        s_bg = ctx.enter_context(nc.semaphore("s_bg"))      # bg broadcast DMA
        s_fin = ctx.enter_context(nc.semaphore("s_fin"))    # DVE final section done
        s_fout = ctx.enter_context(nc.semaphore("s_fout"))  # final output DMAs

        block = ctx.enter_context(nc.Block())

        @block.sync
        def _(sync):
            def load_d(g):
                if g >= IN_BUFS:
                    sync.wait_ge(s_t, g - IN_BUFS + 1)     # d_buf free (scans done)
                d_view = d_bufs[g % IN_BUFS][:].rearrange(
                    "p (j s1) -> p j s1", s1=S + 1
                )[:, :, 1 : S + 1]
                sync.dma_start(out=d_view, in_=d_dram(g)).then_inc(s_ind, 16)

            def load_r(g):
                if g >= IN_BUFS:
                    sync.wait_ge(s_dve, g - IN_BUFS + 1)   # r_buf free (einsum done)
                sync.dma_start(out=r_bufs[g % IN_BUFS][:], in_=r_dram(g)).then_inc(
                    s_inr, 16
                )

            load_d(0)
            load_d(1)
            for g in range(GROUPS - 2):
                load_r(g)
                load_d(g + 2)
            load_r(GROUPS - 2)
            load_r(GROUPS - 1)


        @block.scalar
        def _(scalar):
            for g in range(GROUPS):
                scalar.wait_ge(s_add, g + 1)
                dv = d_bufs[g % IN_BUFS][:].rearrange(
                    "p (j s1) -> p j s1", s1=S + 1
                )[:, :, 1 : S + 1]
                if interval == 1.0:
                    scalar.activation(dv, dv, Act.Sigmoid, scale=-1.0)
                else:
                    scalar.activation(dv, dv, Act.Sigmoid, scale=-1.0)
                    scalar.activation(dv, dv, Act.Ln)
                    scalar.activation(dv, dv, Act.Exp, scale=float(interval))
                scalar.drain().then_inc(s_u, 1)
                if g >= 1:
                    scalar.wait_ge(s_w, g)
                    scalar.dma_start(
                        out=w_dram(g - 1), in_=w_bufs[(g - 1) % W_BUFS][:]
                    ).then_inc(s_wst, 16)
            scalar.wait_ge(s_w, GROUPS)
            scalar.dma_start(
                out=w_dram(GROUPS - 1), in_=w_bufs[(GROUPS - 1) % W_BUFS][:]
            ).then_inc(s_wst, 16)

        @block.gpsimd
        def _(gpsimd):
            gpsimd.dma_start(out=shiftb[:], in_=bcast_ap(shift_h)).then_inc(s_bc, 16)
            gpsimd.dma_start(out=bgb[:], in_=bcast_ap(bg_h)).then_inc(s_bg, 16)
            gpsimd.wait_ge(s_wst, 16 * GROUPS)
            gpsimd.wait_ge(s_fin, 1)
            gpsimd.dma_start(
                out=alphainv_h[:].rearrange("(t p) -> t p", p=P), in_=at_t[:]
            ).then_inc(s_fout, 16)
            gpsimd.dma_start(
                out=rgbm_h[:].rearrange("c (t p) -> t c p", p=P),
                in_=rm_t[:].rearrange("t (c p) -> t c p", p=P),
            ).then_inc(s_fout, 16)
            gpsimd.wait_ge(s_fout, 32)

        @block.vector
        def _(vector):
            def add_group(g):
                xv = d_bufs[g % IN_BUFS][:].rearrange(
                    "p (j s1) -> p j s1", s1=S + 1
                )[:, :, 1 : S + 1]
                vector.tensor_tensor(
                    out=xv,
                    in0=xv,
                    in1=shiftb[:, None, :].broadcast_to([P, G, S]),
                    op=Alu.add,
                ).then_inc(s_add, 1)

            def scans(g):
                # one fused scan: state = u*state + z1; the pad columns
                # (u=0, z1=1) reset state to 1.0 at every segment start
                vector.tensor_tensor_scan(
                    out=t_bufs[g % 2][:],
                    data0=d_bufs[g % IN_BUFS][:],
                    data1=z1[:],
                    initial=1.0,
                    op0=Alu.mult,
                    op1=Alu.add,
                ).then_inc(s_t, 1)

            # one-time constants, written long before first use
            vector.memset(z1[:], 0.0)
            z1v = z1[:].rearrange("p (j s1) -> p j s1", s1=S + 1)
            vector.memset(z1v[:, :, 0], 1.0)
            for i in range(IN_BUFS):
                dv0 = d_bufs[i][:].rearrange("p (j s1) -> p j s1", s1=S + 1)
                vector.memset(dv0[:, :, 0], 0.0)

            vector.wait_ge(s_bc, 16)
            vector.wait_ge(s_ind, 16)
            add_group(0)
            vector.wait_ge(s_u, 1)
            scans(0)

            for g in range(GROUPS):
                if g + 1 < GROUPS:
                    vector.wait_ge(s_ind, 16 * (g + 2))
                    add_group(g + 1)

                # weights = T_excl - T_incl from this group's cumprod
                t_g = t_bufs[g % 2]
                w_g = w_bufs[g % W_BUFS]
                tv = t_g[:].rearrange("p (j s1) -> p j s1", j=G)
                wv = w_g[:].rearrange("p (j s) -> p j s", j=G)
                if g >= W_BUFS:
                    vector.wait_ge(s_wst, 16 * (g - W_BUFS + 1))
                vector.tensor_tensor(
                    out=wv,
                    in0=tv[:, :, 0:S],
                    in1=tv[:, :, 1 : S + 1],
                    op=Alu.subtract,
                )
                vector.tensor_copy(
                    out=abuf[:, g * G : (g + 1) * G], in_=tv[:, :, S]
                )

                if g + 1 < GROUPS:
                    vector.wait_ge(s_u, g + 2)
                    scans(g + 1)

                vector.wait_ge(s_inr, 16 * (g + 1))
                r_g = r_bufs[g % IN_BUFS]
                wv = w_g[:].rearrange("p (j s) -> p j s", j=G)
                rv = r_g[:].rearrange("p (j s c) -> p j s c", j=G, c=3)
                for j in range(G):
                    for c in range(3):
                        t_glob = g * G + j
                        inst = nc.vector.scalar_tensor_tensor(
                            out=scratch[:],
                            in0=wv[:, j, :],
                            scalar=0.0,
                            in1=rv[:, j, :, c],
                            op0=Alu.bypass,
                            op1=Alu.mult,
                            accum_out=rm_all[
                                :, c * TILES + t_glob : c * TILES + t_glob + 1
                            ],
                        )
                        if j == 0 and c == 0:
                            inst.then_inc(s_w, 1)
                        if j == G - 1 and c == 2:
                            inst.then_inc(s_dve, 1)

            vector.wait_ge(s_bg, 16)
            # rgb_marched += alphainv_last * bg  (channel-planar, contiguous)
            for c in range(3):
                nc.vector.scalar_tensor_tensor(
                    out=rm_all[:, c * TILES : (c + 1) * TILES],
                    in0=abuf[:],
                    scalar=bgb[:, c : c + 1],
                    in1=rm_all[:, c * TILES : (c + 1) * TILES],
                    op0=Alu.mult,
                    op1=Alu.add,
                )

            # transpose abuf [P, TILES] -> at_t [TILES, P] in 32x32 blocks
            for i in range(P // B):
                vector.transpose(
                    out=at_t[0:B, i * B : (i + 1) * B],
                    in_=abuf[i * B : (i + 1) * B, :],
                )
            # transpose each channel plane [P, TILES] -> [TILES, P] blockwise
            for c in range(3):
                for i in range(P // B):
                    vector.transpose(
                        out=rm_t[0:B, c * P + i * B : c * P + (i + 1) * B],
                        in_=rm_all[i * B : (i + 1) * B, c * TILES : (c + 1) * TILES],
                    )
            vector.drain().then_inc(s_fin, 1)

    return nc


def kernel(density, shift, rgb, bg, interval):
    global LAST_EXEC_TIME_NS, LAST_RESULT
    from concourse.bass_utils import run_bass_kernel_spmd

    density = np.ascontiguousarray(density, dtype=np.float32)
    shift = np.ascontiguousarray(shift, dtype=np.float32)
    rgb = np.ascontiguousarray(rgb, dtype=np.float32)
    bg = np.ascontiguousarray(bg, dtype=np.float32)
    interval_f = float(np.asarray(interval))

    key = interval_f
    if key not in _program_cache:
        _program_cache[key] = _build_program(interval_f)
    nc = _program_cache[key]

    rgb2 = rgb.reshape(N_RAYS, S * 3)
    in_maps = []
    for i in range(N_CORES):
        rows = slice(i * N_PER_CORE, (i + 1) * N_PER_CORE)
        in_maps.append(
            {
                "density": density[rows],
                "shift": shift,
                "rgb": rgb2[rows],
                "bg": bg,
            }
        )

    res = run_bass_kernel_spmd(nc, in_maps, list(range(N_CORES)))
    LAST_EXEC_TIME_NS = res.exec_time_ns
    LAST_RESULT = res

    rgb_marched = np.concatenate(
        [np.ascontiguousarray(r["rgb_marched_t"].T) for r in res.results], axis=0
    )
    weights = np.concatenate([r["weights"] for r in res.results], axis=0)
    alphainv_last = np.concatenate([r["alphainv_last"] for r in res.results], axis=0)
    return rgb_marched, weights, alphainv_last
